# revision 3
# baseline (speedup 1.0000x reference)
"""GAT layer (nn_GATLayer) on 8 Trainium2 NeuronCores.

Math (reference):
    Wh = X @ weight                      [N, F]
    s  = Wh @ a[:F];  t = Wh @ a[F:]     [N, 1]
    e  = relu(s_i + t_j)                 [N, N]
    att = softmax(where(A > 0, e, -9e15), axis=1)
    out = elu(att @ Wh)

Kernel formulation (shift-free softmax, exact up to fp rounding):
    p_ij  = A_ij * max(exp(s_i + t_j), 1)     (exp(relu(x)) = max(exp(x), 1))
    out_i = elu((p_i: @ Wh) / sum_j p_ij)

Sharding: 1D row partition of A across 8 cores (1024 rows each); X,
weight, a replicated. Each core computes full Wh (needed as the value
matrix by every core).

Per-core dataflow (all natural [i-on-partitions] layout):
  - weight cast-loaded bf16; w_t = weight @ a[F:] via PE (weight
    transposed by 128x128 XPOSE blocks); Wh_aug[jt] = [Wh[jt] | 1]
    built by a PE matmul with X^T blocks (XPOSE'd from cast-loaded X)
    as stationaries and [weight | w_t] as moving -> also yields t.
  - t assembled to a DRAM row, broadcast-cast back as T_bcast [128, N].
  - s for own rows from an X_own mini-matmul.
  - main loop: A cast-loaded to fp8 {0,1}; ACT: z = exp(T_bcast + s_i);
    DVE fused: p = (z max 1) * A; 128x128 XPOSE -> p^T blocks used as
    matmul stationaries vs Wh_aug moving -> PSUM [128, 129] accumulates
    numerators + denominator (ones column).
  - epilogue: reciprocal, scale, ELU composed as
    elu(x) = exp(min(x,0)) - 1 + max(x,0), DMA out.
"""

import numpy as np

import concourse.bass as bass
import concourse.bacc as bacc
import concourse.mybir as mybir
import concourse.tile as tile
from concourse.bass_utils import run_bass_kernel_spmd

N = 8192
F_IN = 512
F_OUT = 128
N_CORES = 8
ROWS = N // N_CORES          # 1024 rows per core
RT = ROWS // 128             # 8 own row tiles
NT = N // 128                # 64 j tiles
NC_CHUNK = 1024              # j-chunk width for elementwise tiles
NCH = N // NC_CHUNK          # 8 chunks
KC = F_IN // 128             # 4 f_in chunks

FP32 = mybir.dt.float32
BF16 = mybir.dt.bfloat16
FP8 = mybir.dt.float8e4
Alu = mybir.AluOpType
Act = mybir.ActivationFunctionType

_cache = {}


def _build():
    nc = bacc.Bacc("TRN2", target_bir_lowering=False, debug=False)

    A_blk = nc.dram_tensor("A_blk", [ROWS, N], mybir.dt.int32, kind="ExternalInput")
    X_full = nc.dram_tensor("X_full", [N, F_IN], FP32, kind="ExternalInput")
    X_own = nc.dram_tensor("X_own", [ROWS, F_IN], FP32, kind="ExternalInput")
    weight = nc.dram_tensor("weight", [F_IN, F_OUT], FP32, kind="ExternalInput")
    a_vec = nc.dram_tensor("a_vec", [2 * F_OUT, 1], FP32, kind="ExternalInput")
    ident = nc.dram_tensor("ident", [128, 128], FP32, kind="ExternalInput")
    out_d = nc.dram_tensor("out", [ROWS, F_OUT], FP32, kind="ExternalOutput")

    with tile.TileContext(nc) as tc:
        with (
            tc.tile_pool(name="setup", bufs=1) as setup,
            tc.tile_pool(name="whaug", bufs=NT) as whaug_pool,
            tc.tile_pool(name="tbc", bufs=NCH) as tbc_pool,
            tc.tile_pool(name="scol", bufs=RT) as scol_pool,
            tc.tile_pool(name="xnat", bufs=4) as xnat_pool,
            tc.tile_pool(name="xt", bufs=8) as xt_pool,
            tc.tile_pool(name="an", bufs=3) as an_pool,
            tc.tile_pool(name="zz", bufs=3) as zz_pool,
            tc.tile_pool(name="pp", bufs=3) as pp_pool,
            tc.tile_pool(name="pt", bufs=8) as pt_pool,
            tc.tile_pool(name="epi", bufs=2) as epi_pool,
            tc.tile_pool(name="psw", bufs=2, space="PSUM") as psw_pool,
            tc.tile_pool(name="pso", bufs=2, space="PSUM") as pso_pool,
            tc.tile_pool(name="dram", bufs=1, space="DRAM") as dram_pool,
        ):
            # ---------------- setup: weight, a, w_t, w_cat ----------------
            idn = setup.tile([128, 128], FP32)
            nc.sync.dma_start(out=idn, in_=ident[:, :])

            a_cat = setup.tile([128, 2], BF16)
            nc.gpsimd.dma_start(out=a_cat[:, 0:1], in_=a_vec[0:F_OUT, :])
            nc.gpsimd.dma_start(out=a_cat[:, 1:2], in_=a_vec[F_OUT:, :])

            # Wgt_aug[k]: [weight chunk bf16 | w_t chunk]  [128, 129]
            wgt_aug = []
            for k in range(KC):
                wa = setup.tile([128, F_OUT + 1], BF16, tag=f"wgt_aug{k}")
                nc.gpsimd.dma_start(
                    out=wa[:, 0:F_OUT], in_=weight[128 * k : 128 * (k + 1), :]
                )
                wgt_aug.append(wa)

            # weight chunks transposed (for w_st = weight.T-chunks @ a)
            w_cat = []
            for k in range(KC):
                wT = setup.tile([128, 128], BF16, tag=f"wT{k}")
                nc.sync.dma_start(out=wT, in_=wgt_aug[k][:, 0:F_OUT], transpose=True)
                ps = psw_pool.tile([128, 2], FP32, tag="ps")
                nc.tensor.matmul(ps, wT, a_cat, start=True, stop=True)
                wc = setup.tile([128, 2], BF16, tag=f"w_cat{k}")
                nc.vector.tensor_copy(wc, ps)
                w_cat.append(wc)
                # place w_t chunk into Wgt_aug col F_OUT
                nc.vector.tensor_copy(wgt_aug[k][:, F_OUT : F_OUT + 1], ps[:, 1:2])

            # ---------------- phase 1: Wh_aug + t ----------------
            t_cols = setup.tile([128, NT], FP32)
            wh_aug = []
            for r in range(NT):
                xn = xnat_pool.tile([128, F_IN], BF16)
                nc.gpsimd.dma_start(out=xn, in_=X_full[128 * r : 128 * (r + 1), :])
                ps = psw_pool.tile([128, F_OUT + 1], FP32, tag="ps")
                for k in range(KC):
                    xt = xt_pool.tile([128, 128], BF16)
                    nc.sync.dma_start(
                        out=xt, in_=xn[:, 128 * k : 128 * (k + 1)], transpose=True
                    )
                    # [Wh | t] chunk-accumulate; moving = [weight_k | w_t_k]
                    nc.tensor.matmul(
                        ps, xt, wgt_aug[k], start=(k == 0), stop=(k == KC - 1)
                    )
                wh = whaug_pool.tile([128, F_OUT + 1], BF16)
                nc.vector.tensor_copy(wh[:, 0:F_OUT], ps[:, 0:F_OUT])
                nc.vector.memset(wh[:, F_OUT : F_OUT + 1], 1.0)
                wh_aug.append(wh)
                nc.vector.tensor_copy(t_cols[:, r : r + 1], ps[:, F_OUT : F_OUT + 1])

            # t_cols [128, NT] -> t_row in DRAM [1, N] -> T_bcast chunks
            ps_t = psw_pool.tile([NT, 128], FP32, tag="ps")
            nc.tensor.transpose(ps_t, t_cols, idn)
            tT = setup.tile([NT, 128], FP32)
            nc.vector.tensor_copy(tT, ps_t)
            t_dram = dram_pool.tile([1, N], FP32)
            nc.sync.dma_start(out=t_dram[:, :], in_=tT)
            t_bc = []
            for c in range(NCH):
                tb = tbc_pool.tile([128, NC_CHUNK], BF16)
                bcast_ap = bass.AP(
                    tensor=t_dram.tensor,
                    offset=t_dram.offset + NC_CHUNK * c,
                    ap=[[0, 128], [1, NC_CHUNK]],
                )
                nc.gpsimd.dma_start(out=tb, in_=bcast_ap)
                t_bc.append(tb)

            # ---------------- phase 1b: s for own rows ----------------
            s_col = []
            for q in range(RT):
                xn = xnat_pool.tile([128, F_IN], BF16)
                nc.gpsimd.dma_start(out=xn, in_=X_own[128 * q : 128 * (q + 1), :])
                ps = psw_pool.tile([128, 2], FP32, tag="ps")
                for k in range(KC):
                    xt = xt_pool.tile([128, 128], BF16)
                    nc.sync.dma_start(
                        out=xt, in_=xn[:, 128 * k : 128 * (k + 1)], transpose=True
                    )
                    nc.tensor.matmul(
                        ps, xt, w_cat[k], start=(k == 0), stop=(k == KC - 1)
                    )
                sc = scol_pool.tile([128, 1], FP32)
                nc.vector.tensor_copy(sc, ps[:, 0:1])
                s_col.append(sc)

            # ---------------- phase 2: attention + aggregate ----------------
            for it in range(RT):
                pso = pso_pool.tile([128, F_OUT + 1], FP32)
                for jc in range(NCH):
                    an = an_pool.tile([128, NC_CHUNK], FP8)
                    nc.gpsimd.dma_start(
                        out=an,
                        in_=A_blk[
                            128 * it : 128 * (it + 1),
                            NC_CHUNK * jc : NC_CHUNK * (jc + 1),
                        ],
                    )
                    z = zz_pool.tile([128, NC_CHUNK], BF16)
                    nc.scalar.activation(
                        out=z, in_=t_bc[jc], func=Act.Exp, bias=s_col[it][:, 0:1]
                    )
                    p = pp_pool.tile([128, NC_CHUNK], BF16)
                    nc.vector.scalar_tensor_tensor(
                        out=p, in0=z, scalar=1.0, in1=an,
                        op0=Alu.max, op1=Alu.mult,
                    )
                    for b in range(NC_CHUNK // 128):
                        jt = (NC_CHUNK // 128) * jc + b
                        pt = pt_pool.tile([128, 128], BF16)
                        nc.sync.dma_start(
                            out=pt, in_=p[:, 128 * b : 128 * (b + 1)], transpose=True
                        )
                        nc.tensor.matmul(
                            pso, pt, wh_aug[jt],
                            start=(jt == 0), stop=(jt == NT - 1),
                        )
                # epilogue: x = num / denom; out = elu(x)
                rec = epi_pool.tile([128, 1], FP32, tag="rec")
                nc.vector.reciprocal(rec, pso[:, F_OUT : F_OUT + 1])
                xsc = epi_pool.tile([128, F_OUT], FP32, tag="xsc")
                nc.vector.tensor_scalar(
                    out=xsc, in0=pso[:, 0:F_OUT], scalar1=rec[:, 0:1], scalar2=None,
                    op0=Alu.mult,
                )
                m0 = epi_pool.tile([128, F_OUT], FP32, tag="m0")
                nc.vector.tensor_scalar(
                    out=m0, in0=xsc, scalar1=0.0, scalar2=None, op0=Alu.min
                )
                e0 = epi_pool.tile([128, F_OUT], FP32, tag="e0")
                nc.scalar.activation(out=e0, in_=m0, func=Act.Exp)
                r0 = epi_pool.tile([128, F_OUT], FP32, tag="r0")
                nc.vector.tensor_scalar(
                    out=r0, in0=xsc, scalar1=0.0, scalar2=None, op0=Alu.max
                )
                ot = epi_pool.tile([128, F_OUT], FP32, tag="ot")
                nc.vector.scalar_tensor_tensor(
                    out=ot, in0=e0, scalar=-1.0, in1=r0, op0=Alu.add, op1=Alu.add
                )
                nc.sync.dma_start(
                    out=out_d[128 * it : 128 * (it + 1), :], in_=ot
                )

    nc.compile()
    return nc


def kernel(X, A, weight, a, _trace=False, _tmpdir=None):
    X = np.ascontiguousarray(np.asarray(X, dtype=np.float32))
    A = np.ascontiguousarray(np.asarray(A, dtype=np.int32))
    weight = np.ascontiguousarray(np.asarray(weight, dtype=np.float32))
    a = np.ascontiguousarray(np.asarray(a, dtype=np.float32))

    if "nc" not in _cache:
        _cache["nc"] = _build()
    nc = _cache["nc"]

    ident = np.eye(128, dtype=np.float32)
    in_maps = []
    for c in range(N_CORES):
        i0 = c * ROWS
        in_maps.append(
            {
                "A_blk": A[i0 : i0 + ROWS],
                "X_full": X,
                "X_own": X[i0 : i0 + ROWS],
                "weight": weight,
                "a_vec": a,
                "ident": ident,
            }
        )

    res = run_bass_kernel_spmd(
        nc, in_maps, core_ids=list(range(N_CORES)), trace=_trace, tmpdir=_tmpdir
    )
    out = np.concatenate([res.results[c]["out"] for c in range(N_CORES)], axis=0)
    if _trace:
        kernel._last_results = res
    return out


# revision 5
# speedup vs baseline: 5.1322x; 5.1322x over previous
"""GAT layer (nn_GATLayer) on 8 Trainium2 NeuronCores.

Math (reference):
    Wh = X @ weight                      [N, F]
    s  = Wh @ a[:F];  t = Wh @ a[F:]     [N, 1]
    e  = relu(s_i + t_j)                 [N, N]
    att = softmax(where(A > 0, e, -9e15), axis=1)
    out = elu(att @ Wh)

Kernel formulation (shift-free softmax, exact up to fp rounding):
    p_ij  = A_ij * max(exp(s_i + t_j), 1)     (exp(relu(x)) = max(exp(x), 1))
    out_i = elu((p_i: @ Wh) / sum_j p_ij)

Sharding: 1D row partition of A across 8 cores (1024 rows each); X,
weight, a replicated; out rows gathered on host.

Per-core dataflow (v2: transposed orientation [j, i]; the only large
transpose is A's, as 64 big DRAM->SBUF xbar DMAs):
  - A_blk int32 -> bf16 {0,1.0} via SWDGE DRAM->DRAM cast (8 chunks),
    then 64 DmaTranspose [1024, 128] -> at_slab [128 j, 1024 i].
  - X -> bf16 DRAM (D2D cast), 4 big transposes -> X^T chunks; Wh_nat
    [128 j, 128 f] + t columns from PE (stationary = X^T slice, moving
    = [weight | w_t]); w_t = weight.T-chunks @ a via tiny PE matmuls.
  - s (own rows) from an X_own mini-matmul, assembled into a DRAM row,
    broadcast-cast back as S_bcast [128, 1024 i].
  - main loop over 64 j-tiles: ACT z = exp(S_bcast + t_j); one fused
    DVE op p^T = (z max 1) * at_slab; PE: psum_oT [128 f, 1024 i] +=
    contraction of Wh_nat[jt] with p^T (N=512 x2), denominator row via
    ones stationary.
  - epilogue: reciprocal of denom -> DRAM broadcast -> scale, ELU
    (exp(min(x,0)) - 1 + max(x,0)), 8 PE transposes back to natural,
    DMA out.
"""

import numpy as np

import concourse.bass as bass
import concourse.bacc as bacc
import concourse.mybir as mybir
import concourse.tile as tile
from concourse.bass_utils import run_bass_kernel_spmd

N = 8192
F_IN = 512
F_OUT = 128
N_CORES = 8
ROWS = N // N_CORES          # 1024 rows per core
RT = ROWS // 128             # 8 own row tiles
NT = N // 128                # 64 j tiles
KC = F_IN // 128             # 4 f_in chunks
DCH = 8                      # A cast D2D chunks

FP32 = mybir.dt.float32
BF16 = mybir.dt.bfloat16
Alu = mybir.AluOpType
Act = mybir.ActivationFunctionType

_cache = {}


def _build():
    nc = bacc.Bacc("TRN2", target_bir_lowering=False, debug=False)

    A_blk = nc.dram_tensor("A_blk", [ROWS, N], mybir.dt.int32, kind="ExternalInput")
    X_full = nc.dram_tensor("X_full", [N, F_IN], FP32, kind="ExternalInput")
    X_own = nc.dram_tensor("X_own", [ROWS, F_IN], FP32, kind="ExternalInput")
    weight = nc.dram_tensor("weight", [F_IN, F_OUT], FP32, kind="ExternalInput")
    a_vec = nc.dram_tensor("a_vec", [2 * F_OUT, 1], FP32, kind="ExternalInput")
    ident = nc.dram_tensor("ident", [128, 128], FP32, kind="ExternalInput")
    out_d = nc.dram_tensor("out", [ROWS, F_OUT], FP32, kind="ExternalOutput")

    with tile.TileContext(nc) as tc:
        with tc.tile_pool(name="dram", bufs=1, space="DRAM") as dram_pool:
            A_bf = dram_pool.tile([ROWS, N], BF16)
            X_bf = dram_pool.tile([N, F_IN], BF16)
            Xo_bf = dram_pool.tile([ROWS, F_IN], BF16)
            s_dram = dram_pool.tile([1, ROWS], FP32)
            r_dram = dram_pool.tile([1, ROWS], FP32)

            # ---- D2D casts (SWDGE): A int32 -> bf16, X f32 -> bf16 ----
            for c in range(DCH):
                w = N // DCH
                nc.gpsimd.dma_start(
                    out=A_bf[:, w * c : w * (c + 1)],
                    in_=A_blk[:, w * c : w * (c + 1)],
                )
            for c in range(4):
                w = N // 4
                nc.gpsimd.dma_start(
                    out=X_bf[w * c : w * (c + 1), :],
                    in_=X_full[w * c : w * (c + 1), :],
                )
            nc.gpsimd.dma_start(out=Xo_bf[:, :], in_=X_own[:, :])

            with (
                tc.tile_pool(name="setup", bufs=1) as setup,
                tc.tile_pool(name="whn", bufs=NT) as whn_pool,
                tc.tile_pool(name="slab", bufs=6) as slab_pool,
                tc.tile_pool(name="zz", bufs=3) as zz_pool,
                tc.tile_pool(name="pp", bufs=3) as pp_pool,
                tc.tile_pool(name="epi", bufs=2) as epi_pool,
                tc.tile_pool(name="psA", bufs=2, space="PSUM") as psA,
            ):
                # ---------------- setup ----------------
                idn = setup.tile([128, 128], FP32)
                nc.sync.dma_start(out=idn, in_=ident[:, :])
                ones_c = setup.tile([128, 1], BF16)
                nc.vector.memset(ones_c, 1.0)

                a_cat = setup.tile([128, 2], BF16)
                nc.gpsimd.dma_start(out=a_cat[:, 0:1], in_=a_vec[0:F_OUT, :])
                nc.gpsimd.dma_start(out=a_cat[:, 1:2], in_=a_vec[F_OUT:, :])

                # Wgt_aug[k] = [weight_k bf16 | w_t_k], w_cat[k] = [w_s|w_t]
                wgt_aug = []
                for k in range(KC):
                    wa = setup.tile([128, F_OUT + 1], BF16, tag=f"wgt_aug{k}")
                    nc.gpsimd.dma_start(
                        out=wa[:, 0:F_OUT], in_=weight[128 * k : 128 * (k + 1), :]
                    )
                    wgt_aug.append(wa)
                w_cat = []
                for k in range(KC):
                    wT = setup.tile([128, 128], BF16, tag=f"wT{k}")
                    nc.sync.dma_start(
                        out=wT, in_=wgt_aug[k][:, 0:F_OUT], transpose=True
                    )
                    ps = psA.tile([128, 2], FP32, tag="ps")
                    nc.tensor.matmul(ps, wT, a_cat, start=True, stop=True)
                    wc = setup.tile([128, 2], BF16, tag=f"w_cat{k}")
                    nc.vector.tensor_copy(wc, ps)
                    w_cat.append(wc)
                    nc.vector.tensor_copy(
                        wgt_aug[k][:, F_OUT : F_OUT + 1], ps[:, 1:2]
                    )

                # ---------------- X^T chunks (big xposes) ----------------
                xT = []
                for k in range(KC):
                    xt = setup.tile([128, N], BF16, tag=f"xT{k}")
                    nc.sync.dma_start(
                        out=xt, in_=X_bf[:, 128 * k : 128 * (k + 1)], transpose=True
                    )
                    xT.append(xt)
                xoT = []
                for k in range(KC):
                    xt = setup.tile([128, ROWS], BF16, tag=f"xoT{k}")
                    nc.sync.dma_start(
                        out=xt, in_=Xo_bf[:, 128 * k : 128 * (k + 1)], transpose=True
                    )
                    xoT.append(xt)

                # ---------------- Wh_nat + t_cols ----------------
                t_cols = setup.tile([128, NT], FP32)
                wh_nat = []
                for r in range(NT):
                    ps = psA.tile([128, F_OUT + 1], FP32, tag="ps")
                    for k in range(KC):
                        nc.tensor.matmul(
                            ps,
                            xT[k][:, 128 * r : 128 * (r + 1)],
                            wgt_aug[k],
                            start=(k == 0),
                            stop=(k == KC - 1),
                        )
                    wh = whn_pool.tile([128, F_OUT], BF16)
                    nc.vector.tensor_copy(wh, ps[:, 0:F_OUT])
                    wh_nat.append(wh)
                    nc.vector.tensor_copy(
                        t_cols[:, r : r + 1], ps[:, F_OUT : F_OUT + 1]
                    )

                # ---------------- s (own rows) -> S_bcast ----------------
                s_cols = setup.tile([128, RT], FP32)
                for q in range(RT):
                    ps = psA.tile([128, 2], FP32, tag="ps")
                    for k in range(KC):
                        nc.tensor.matmul(
                            ps,
                            xoT[k][:, 128 * q : 128 * (q + 1)],
                            w_cat[k],
                            start=(k == 0),
                            stop=(k == KC - 1),
                        )
                    nc.vector.tensor_copy(s_cols[:, q : q + 1], ps[:, 0:1])
                ps_sT = psA.tile([RT, 128], FP32, tag="ps")
                nc.tensor.transpose(ps_sT, s_cols, idn)
                sT = setup.tile([RT, 128], FP32)
                nc.vector.tensor_copy(sT, ps_sT)
                nc.sync.dma_start(out=s_dram[:, :], in_=sT)
                s_bc = setup.tile([128, ROWS], FP32)
                nc.gpsimd.dma_start(
                    out=s_bc,
                    in_=bass.AP(
                        tensor=s_dram.tensor, offset=s_dram.offset,
                        ap=[[0, 128], [1, ROWS]],
                    ),
                )

                # ---------------- A^T slabs + main loop ----------------
                with (
                    tc.tile_pool(name="psO", bufs=1, space="PSUM") as psO,
                    tc.tile_pool(name="psD", bufs=1, space="PSUM") as psD,
                ):
                    ps_oT = psO.tile([128, ROWS], FP32)
                    ps_d = psD.tile([1, ROWS], FP32)
                    for jt in range(NT):
                        at = slab_pool.tile([128, ROWS], BF16)
                        nc.sync.dma_start(
                            out=at, in_=A_bf[:, 128 * jt : 128 * (jt + 1)],
                            transpose=True,
                        )
                        z = zz_pool.tile([128, ROWS], BF16)
                        nc.scalar.activation(
                            out=z, in_=s_bc, func=Act.Exp,
                            bias=t_cols[:, jt : jt + 1],
                        )
                        p = pp_pool.tile([128, ROWS], BF16)
                        nc.vector.scalar_tensor_tensor(
                            out=p, in0=z, scalar=1.0, in1=at,
                            op0=Alu.max, op1=Alu.mult,
                        )
                        first, last = jt == 0, jt == NT - 1
                        for h in range(2):
                            sl = slice(512 * h, 512 * (h + 1))
                            nc.tensor.matmul(
                                ps_oT[:, sl], wh_nat[jt], p[:, sl],
                                start=first, stop=last, skip_group_check=True,
                            )
                            nc.tensor.matmul(
                                ps_d[:, sl], ones_c, p[:, sl],
                                start=first, stop=last, skip_group_check=True,
                            )

                    # ---------------- epilogue ----------------
                    rec = epi_pool.tile([1, ROWS], FP32, tag="rec")
                    nc.vector.reciprocal(rec, ps_d)
                    nc.sync.dma_start(out=r_dram[:, :], in_=rec)
                    r_bc = epi_pool.tile([128, ROWS], FP32, tag="r_bc")
                    nc.gpsimd.dma_start(
                        out=r_bc,
                        in_=bass.AP(
                            tensor=r_dram.tensor, offset=r_dram.offset,
                            ap=[[0, 128], [1, ROWS]],
                        ),
                    )
                    xsc = epi_pool.tile([128, ROWS], FP32, tag="xsc")
                    nc.vector.tensor_tensor(
                        out=xsc, in0=ps_oT, in1=r_bc, op=Alu.mult
                    )
                    # ELU = exp(min(x,0)) - 1 + max(x,0)
                    m0 = epi_pool.tile([128, ROWS], FP32, tag="m0")
                    nc.vector.tensor_scalar(
                        out=m0, in0=xsc, scalar1=0.0, scalar2=None, op0=Alu.min
                    )
                    e0 = epi_pool.tile([128, ROWS], FP32, tag="e0")
                    nc.scalar.activation(out=e0, in_=m0, func=Act.Exp)
                    r0 = epi_pool.tile([128, ROWS], FP32, tag="r0")
                    nc.vector.tensor_scalar(
                        out=r0, in0=xsc, scalar1=0.0, scalar2=None, op0=Alu.max
                    )
                    oT = epi_pool.tile([128, ROWS], FP32, tag="oT")
                    nc.vector.scalar_tensor_tensor(
                        out=oT, in0=e0, scalar=-1.0, in1=r0,
                        op0=Alu.add, op1=Alu.add,
                    )
                    for q in range(RT):
                        ps_f = psA.tile([128, 128], FP32, tag="ps")
                        nc.tensor.transpose(
                            ps_f, oT[:, 128 * q : 128 * (q + 1)], idn
                        )
                        of = epi_pool.tile([128, F_OUT], FP32, tag="of")
                        nc.scalar.copy(of, ps_f)
                        nc.sync.dma_start(
                            out=out_d[128 * q : 128 * (q + 1), :], in_=of
                        )

    nc.compile()
    return nc


def kernel(X, A, weight, a, _trace=False, _tmpdir=None):
    X = np.ascontiguousarray(np.asarray(X, dtype=np.float32))
    A = np.ascontiguousarray(np.asarray(A, dtype=np.int32))
    weight = np.ascontiguousarray(np.asarray(weight, dtype=np.float32))
    a = np.ascontiguousarray(np.asarray(a, dtype=np.float32))

    if "nc" not in _cache:
        _cache["nc"] = _build()
    nc = _cache["nc"]

    ident = np.eye(128, dtype=np.float32)
    in_maps = []
    for c in range(N_CORES):
        i0 = c * ROWS
        in_maps.append(
            {
                "A_blk": A[i0 : i0 + ROWS],
                "X_full": X,
                "X_own": X[i0 : i0 + ROWS],
                "weight": weight,
                "a_vec": a,
                "ident": ident,
            }
        )

    res = run_bass_kernel_spmd(
        nc, in_maps, core_ids=list(range(N_CORES)), trace=_trace, tmpdir=_tmpdir
    )
    out = np.concatenate([res.results[c]["out"] for c in range(N_CORES)], axis=0)
    if _trace:
        kernel._last_results = res
    return out


# revision 7
# speedup vs baseline: 201.8598x; 39.3324x over previous
"""GAT layer (nn_GATLayer) on 8 Trainium2 NeuronCores.

Math (reference):
    Wh = X @ weight                      [N, F]
    s  = Wh @ a[:F];  t = Wh @ a[F:]     [N, 1]
    e  = relu(s_i + t_j)                 [N, N]
    att = softmax(where(A > 0, e, -9e15), axis=1)
    out = elu(att @ Wh)

Kernel formulation (shift-free softmax, exact up to fp rounding):
    p_ij  = A_ij * max(exp(s_i + t_j), 1)     (exp(relu(x)) = max(exp(x), 1))
    out_i = elu((p_i: @ Wh) / sum_j p_ij)

Sharding: 1D row partition of A across 8 cores (1024 rows each); X,
weight, a replicated; out rows gathered on host.

Per-core dataflow (v2: transposed orientation [j, i]; the only large
transpose is A's, as 64 big DRAM->SBUF xbar DMAs):
  - A_blk int32 -> bf16 {0,1.0} via SWDGE DRAM->DRAM cast (8 chunks),
    then 64 DmaTranspose [1024, 128] -> at_slab [128 j, 1024 i].
  - X -> bf16 DRAM (D2D cast), 4 big transposes -> X^T chunks; Wh_nat
    [128 j, 128 f] + t columns from PE (stationary = X^T slice, moving
    = [weight | w_t]); w_t = weight.T-chunks @ a via tiny PE matmuls.
  - s (own rows) from an X_own mini-matmul, assembled into a DRAM row,
    broadcast-cast back as S_bcast [128, 1024 i].
  - main loop over 64 j-tiles: ACT z = exp(S_bcast + t_j); one fused
    DVE op p^T = (z max 1) * at_slab; PE: psum_oT [128 f, 1024 i] +=
    contraction of Wh_nat[jt] with p^T (N=512 x2), denominator row via
    ones stationary.
  - epilogue: reciprocal of denom -> DRAM broadcast -> scale, ELU
    (exp(min(x,0)) - 1 + max(x,0)), 8 PE transposes back to natural,
    DMA out.
"""

import numpy as np

import concourse.bass as bass
import concourse.bacc as bacc
import concourse.mybir as mybir
import concourse.tile as tile
from concourse.bass_utils import run_bass_kernel_spmd

N = 8192
F_IN = 512
F_OUT = 128
N_CORES = 8
ROWS = N // N_CORES          # 1024 rows per core
RT = ROWS // 128             # 8 own row tiles
NT = N // 128                # 64 j tiles
KC = F_IN // 128             # 4 f_in chunks
DCH = 8                      # A cast D2D chunks

FP32 = mybir.dt.float32
BF16 = mybir.dt.bfloat16
Alu = mybir.AluOpType
Act = mybir.ActivationFunctionType

_cache = {}


def _build():
    nc = bacc.Bacc("TRN2", target_bir_lowering=False, debug=False)

    A_blk = nc.dram_tensor("A_blk", [ROWS, N], mybir.dt.int32, kind="ExternalInput")
    X_own = nc.dram_tensor("X_own", [ROWS, F_IN], FP32, kind="ExternalInput")
    weight = nc.dram_tensor("weight", [F_IN, F_OUT], FP32, kind="ExternalInput")
    a_vec = nc.dram_tensor("a_vec", [2 * F_OUT, 1], FP32, kind="ExternalInput")
    ident = nc.dram_tensor("ident", [128, 128], FP32, kind="ExternalInput")
    out_d = nc.dram_tensor("out", [ROWS, F_OUT], FP32, kind="ExternalOutput")

    with tile.TileContext(nc) as tc:
        with tc.tile_pool(name="dram", bufs=1, space="DRAM") as dram_pool:
            A_bf = dram_pool.tile([ROWS, N], BF16)
            Xo_bf = dram_pool.tile([ROWS, F_IN], BF16)
            s_dram = dram_pool.tile([1, ROWS], FP32)
            r_dram = dram_pool.tile([1, ROWS], FP32)
            cc_in_wh = nc.dram_tensor("cc_in_wh", [ROWS, F_OUT], BF16)
            cc_out_wh = nc.dram_tensor(
                "cc_out_wh", [N, F_OUT], BF16, addr_space="Shared"
            )
            cc_in_t = nc.dram_tensor("cc_in_t", [RT, 128], FP32)
            cc_out_t = nc.dram_tensor(
                "cc_out_t", [NT, 128], FP32, addr_space="Shared"
            )

            # ---- D2D casts (SWDGE): A int32 -> bf16, X f32 -> bf16 ----
            for c in range(DCH):
                w = N // DCH
                nc.gpsimd.dma_start(
                    out=A_bf[:, w * c : w * (c + 1)],
                    in_=A_blk[:, w * c : w * (c + 1)],
                )
            nc.gpsimd.dma_start(out=Xo_bf[:, :], in_=X_own[:, :])

            with (
                tc.tile_pool(name="setup", bufs=1) as setup,
                tc.tile_pool(name="whn", bufs=NT) as whn_pool,
                tc.tile_pool(name="slab", bufs=6) as slab_pool,
                tc.tile_pool(name="zz", bufs=3) as zz_pool,
                tc.tile_pool(name="pp", bufs=3) as pp_pool,
                tc.tile_pool(name="epi", bufs=2) as epi_pool,
                tc.tile_pool(name="psA", bufs=2, space="PSUM") as psA,
            ):
                # ---------------- setup ----------------
                idn = setup.tile([128, 128], FP32)
                nc.sync.dma_start(out=idn, in_=ident[:, :])
                ones_c = setup.tile([128, 1], BF16)
                nc.vector.memset(ones_c, 1.0)

                a_cat = setup.tile([128, 2], BF16)
                nc.gpsimd.dma_start(out=a_cat[:, 0:1], in_=a_vec[0:F_OUT, :])
                nc.gpsimd.dma_start(out=a_cat[:, 1:2], in_=a_vec[F_OUT:, :])

                # w_all[k] = [weight_k bf16 | w_t_k | w_s_k]  [128, 130]
                w_all = []
                for k in range(KC):
                    wa = setup.tile([128, F_OUT + 2], BF16, tag=f"w_all{k}")
                    nc.gpsimd.dma_start(
                        out=wa[:, 0:F_OUT], in_=weight[128 * k : 128 * (k + 1), :]
                    )
                    w_all.append(wa)
                for k in range(KC):
                    wT = setup.tile([128, 128], BF16, tag=f"wT{k}")
                    nc.sync.dma_start(
                        out=wT, in_=w_all[k][:, 0:F_OUT], transpose=True
                    )
                    ps = psA.tile([128, 2], FP32, tag="ps")
                    nc.tensor.matmul(ps, wT, a_cat, start=True, stop=True)
                    # col F_OUT = w_t (a[F:]), col F_OUT+1 = w_s (a[:F])
                    nc.vector.tensor_copy(
                        w_all[k][:, F_OUT : F_OUT + 1], ps[:, 1:2]
                    )
                    nc.vector.tensor_copy(
                        w_all[k][:, F_OUT + 1 : F_OUT + 2], ps[:, 0:1]
                    )

                # ---------------- X_own^T chunks ----------------
                xoT = []
                for k in range(KC):
                    xt = setup.tile([128, ROWS], BF16, tag=f"xoT{k}")
                    nc.sync.dma_start(
                        out=xt, in_=Xo_bf[:, 128 * k : 128 * (k + 1)], transpose=True
                    )
                    xoT.append(xt)

                # ------- own rows: [Wh | t | s] = Xo^T.T-contract @ w_all -------
                s_cols = setup.tile([128, RT], FP32)
                t_own = setup.tile([128, RT], FP32)
                for q in range(RT):
                    ps = psA.tile([128, F_OUT + 2], FP32, tag="ps")
                    for k in range(KC):
                        nc.tensor.matmul(
                            ps,
                            xoT[k][:, 128 * q : 128 * (q + 1)],
                            w_all[k],
                            start=(k == 0),
                            stop=(k == KC - 1),
                        )
                    wh = epi_pool.tile([128, F_OUT], BF16, tag="wh_own")
                    nc.vector.tensor_copy(wh, ps[:, 0:F_OUT])
                    nc.sync.dma_start(
                        out=cc_in_wh[128 * q : 128 * (q + 1), :], in_=wh
                    )
                    nc.vector.tensor_copy(
                        t_own[:, q : q + 1], ps[:, F_OUT : F_OUT + 1]
                    )
                    nc.vector.tensor_copy(
                        s_cols[:, q : q + 1], ps[:, F_OUT + 1 : F_OUT + 2]
                    )
                # s -> DRAM row -> broadcast
                ps_sT = psA.tile([RT, 128], FP32, tag="ps")
                nc.tensor.transpose(ps_sT, s_cols, idn)
                sT = setup.tile([RT, 128], FP32)
                nc.vector.tensor_copy(sT, ps_sT)
                nc.sync.dma_start(out=s_dram[:, :], in_=sT)
                s_bc = setup.tile([128, ROWS], FP32)
                nc.gpsimd.dma_start(
                    out=s_bc,
                    in_=bass.AP(
                        tensor=s_dram.tensor, offset=s_dram.offset,
                        ap=[[0, 128], [1, ROWS]],
                    ),
                )
                # t-own -> [RT, 128] tile-major -> gather
                ps_tT = psA.tile([RT, 128], FP32, tag="ps")
                nc.tensor.transpose(ps_tT, t_own, idn)
                tT = setup.tile([RT, 128], FP32)
                nc.vector.tensor_copy(tT, ps_tT)
                nc.sync.dma_start(out=cc_in_t[:, :], in_=tT)

                # ------- AllGather Wh + t across cores -------
                nc.gpsimd.collective_compute(
                    "AllGather", Alu.bypass,
                    replica_groups=[list(range(N_CORES))],
                    ins=[cc_in_wh[:, :]], outs=[cc_out_wh[:, :]],
                )
                nc.gpsimd.collective_compute(
                    "AllGather", Alu.bypass,
                    replica_groups=[list(range(N_CORES))],
                    ins=[cc_in_t[:, :]], outs=[cc_out_t[:, :]],
                )
                wh_nat = []
                for r in range(NT):
                    wh = whn_pool.tile([128, F_OUT], BF16)
                    nc.sync.dma_start(
                        out=wh, in_=cc_out_wh[128 * r : 128 * (r + 1), :]
                    )
                    wh_nat.append(wh)
                tg = setup.tile([NT, 128], FP32)
                nc.sync.dma_start(out=tg, in_=cc_out_t[:, :])
                ps_tc = psA.tile([128, NT], FP32, tag="ps")
                nc.tensor.transpose(ps_tc, tg, idn[0:NT, 0:NT])
                t_cols = setup.tile([128, NT], FP32)
                nc.vector.tensor_copy(t_cols, ps_tc)

                # ---------------- A^T slabs + main loop ----------------
                with (
                    tc.tile_pool(name="psO", bufs=1, space="PSUM") as psO,
                    tc.tile_pool(name="psD", bufs=1, space="PSUM") as psD,
                ):
                    ps_oT = psO.tile([128, ROWS], FP32)
                    ps_d = psD.tile([1, ROWS], FP32)
                    for jt in range(NT):
                        at = slab_pool.tile([128, ROWS], BF16)
                        nc.sync.dma_start(
                            out=at, in_=A_bf[:, 128 * jt : 128 * (jt + 1)],
                            transpose=True,
                        )
                        z = zz_pool.tile([128, ROWS], BF16)
                        nc.scalar.activation(
                            out=z, in_=s_bc, func=Act.Exp,
                            bias=t_cols[:, jt : jt + 1],
                        )
                        p = pp_pool.tile([128, ROWS], BF16)
                        nc.vector.scalar_tensor_tensor(
                            out=p, in0=z, scalar=1.0, in1=at,
                            op0=Alu.max, op1=Alu.mult,
                        )
                        first, last = jt == 0, jt == NT - 1
                        for h in range(2):
                            sl = slice(512 * h, 512 * (h + 1))
                            nc.tensor.matmul(
                                ps_oT[:, sl], wh_nat[jt], p[:, sl],
                                start=first, stop=last, skip_group_check=True,
                            )
                            nc.tensor.matmul(
                                ps_d[:, sl], ones_c, p[:, sl],
                                start=first, stop=last, skip_group_check=True,
                            )

                    # ---------------- epilogue ----------------
                    rec = epi_pool.tile([1, ROWS], FP32, tag="rec")
                    nc.vector.reciprocal(rec, ps_d)
                    nc.sync.dma_start(out=r_dram[:, :], in_=rec)
                    r_bc = epi_pool.tile([128, ROWS], FP32, tag="r_bc")
                    nc.gpsimd.dma_start(
                        out=r_bc,
                        in_=bass.AP(
                            tensor=r_dram.tensor, offset=r_dram.offset,
                            ap=[[0, 128], [1, ROWS]],
                        ),
                    )
                    xsc = epi_pool.tile([128, ROWS], FP32, tag="xsc")
                    nc.vector.tensor_tensor(
                        out=xsc, in0=ps_oT, in1=r_bc, op=Alu.mult
                    )
                    # ELU = exp(min(x,0)) - 1 + max(x,0)
                    m0 = epi_pool.tile([128, ROWS], FP32, tag="m0")
                    nc.vector.tensor_scalar(
                        out=m0, in0=xsc, scalar1=0.0, scalar2=None, op0=Alu.min
                    )
                    e0 = epi_pool.tile([128, ROWS], FP32, tag="e0")
                    nc.scalar.activation(out=e0, in_=m0, func=Act.Exp)
                    r0 = epi_pool.tile([128, ROWS], FP32, tag="r0")
                    nc.vector.tensor_scalar(
                        out=r0, in0=xsc, scalar1=0.0, scalar2=None, op0=Alu.max
                    )
                    oT = epi_pool.tile([128, ROWS], FP32, tag="oT")
                    nc.vector.scalar_tensor_tensor(
                        out=oT, in0=e0, scalar=-1.0, in1=r0,
                        op0=Alu.add, op1=Alu.add,
                    )
                    for q in range(RT):
                        ps_f = psA.tile([128, 128], FP32, tag="ps")
                        nc.tensor.transpose(
                            ps_f, oT[:, 128 * q : 128 * (q + 1)], idn
                        )
                        of = epi_pool.tile([128, F_OUT], FP32, tag="of")
                        nc.scalar.copy(of, ps_f)
                        nc.sync.dma_start(
                            out=out_d[128 * q : 128 * (q + 1), :], in_=of
                        )

    nc.compile()
    return nc


def kernel(X, A, weight, a, _trace=False, _tmpdir=None):
    X = np.ascontiguousarray(np.asarray(X, dtype=np.float32))
    A = np.ascontiguousarray(np.asarray(A, dtype=np.int32))
    weight = np.ascontiguousarray(np.asarray(weight, dtype=np.float32))
    a = np.ascontiguousarray(np.asarray(a, dtype=np.float32))

    if "nc" not in _cache:
        _cache["nc"] = _build()
    nc = _cache["nc"]

    ident = np.eye(128, dtype=np.float32)
    in_maps = []
    for c in range(N_CORES):
        i0 = c * ROWS
        in_maps.append(
            {
                "A_blk": A[i0 : i0 + ROWS],
                "X_full": X,
                "X_own": X[i0 : i0 + ROWS],
                "weight": weight,
                "a_vec": a,
                "ident": ident,
            }
        )

    res = run_bass_kernel_spmd(
        nc, in_maps, core_ids=list(range(N_CORES)), trace=_trace, tmpdir=_tmpdir
    )
    out = np.concatenate([res.results[c]["out"] for c in range(N_CORES)], axis=0)
    if _trace:
        kernel._last_results = res
    return out
